# revision 32
# baseline (speedup 1.0000x reference)
"""Trainium2 Bass kernel for nn_KVMem (scatter_memory attention-to-memory).

Computation (per reference):
  q = h.reshape(B,S,8,128); k = keys_w.reshape(32768,8,128)
  w = softmax(einsum('bshd,zhd->bshz', q, k), axis=z)
  out = einsum('bshz,hdz->bshd', w, values_w.reshape(8,128,32768))

Strategy: shard the memory axis z (32768) across 8 cores (4096 each).
Each core computes, per head:
  S^T[z,tok] = K^T(tile).T @ Q^T          (TensorE, bf16)
  P^T = exp(S^T)                          (ScalarE, PSUM->SBUF bf16)
  O[tok, 0:128] += P^T(tok-tile).T @ V^T_aug[z, 0:129]  (TensorE, PSUM accum)
where V^T_aug has a 129th column of ones, so column 128 of the O
accumulator is sum_z exp(S) — the softmax denominator — for free.
Host sums partial (O, denom) over cores and divides.

No max-subtraction: scores are q.k with k ~ N(0, 1/1024) over d=128, so
|score| < ~3; exp is safely in fp32/bf16 range.

Performance structure (per core, ~247us vs 317us for the naive overlap):
- TensorE is the roofline: 2056 streamed columns per z-tile (2x512 score
  + 8x129 value MMs) x 256 z-tiles = 227us busy at 2.4GHz. fp8 DoubleRow
  would halve the value phase but measures 3.8% rel err (gate 2e-2).
- exp would serialize everything on ScalarE alone (256 x 1.1us = 294us),
  so each z-tile's scores are computed as two [128,512] single-bank PSUM
  tiles and the two exp halves alternate between ScalarE ACTIVATE and a
  VectorE Schraudolph bit-trick exp: i16 = round(s*128/ln2 + 16250.5) is
  the bf16 bit pattern of ~exp(s)*const; the constant bias cancels in
  the softmax division. 50/50 split -> each engine ~65% busy; ripple
  error ~1.8% on half the tiles -> 1.37e-2 overall (gate 2e-2).
- software pipelining: score MMs run 2 z-tiles ahead of the value MMs
  (5 PSUM score banks in flight), hiding the ~1us exp chain latency; the
  value MMs wait on the exp semaphores with <1% TensorE idle.
- per-engine p-half pools avoid cross-engine WAW coupling; o_ps->SBUF
  copies split over ScalarE/DVE; per-bank output DMAs shorten the tail;
  dummy warmup MMs during the initial DMA wait un-throttle the PE clock
  (HAM) before real work arrives.
"""

import sys

sys.path.insert(0, "/opt/trn_rl_repo")

import numpy as np
import ml_dtypes

NCORES = 8
MEMDIM, MEMSIZE, NHEADS = 1024, 32768, 8
B, S = 2, 512
TOK = B * S  # 1024
HD = MEMDIM // NHEADS  # 128
ZL = MEMSIZE // NCORES  # 4096 z per core
ZT = ZL // 128  # 32 z-tiles per core
TT = TOK // 128  # 8 token tiles
NA = HD + 1  # 129 = value dims + ones column

_compiled = None


def _build():
    import concourse.bass as bass
    import concourse.tile as tile
    from concourse import bacc, mybir

    nc = bacc.Bacc(
        "TRN2", target_bir_lowering=False, debug=False, num_devices=NCORES
    )
    bf16 = mybir.dt.bfloat16
    f32 = mybir.dt.float32

    qT = nc.dram_tensor("qT", [NHEADS, HD, TOK], bf16, kind="ExternalInput").ap()
    kT = nc.dram_tensor("kT", [NHEADS, HD, ZL], bf16, kind="ExternalInput").ap()
    vA = nc.dram_tensor(
        "vA", [NHEADS, 128, ZT * NA], bf16, kind="ExternalInput"
    ).ap()
    outp = nc.dram_tensor(
        "outp", [NHEADS, 128, TT * NA], f32, kind="ExternalOutput"
    ).ap()

    with tile.TileContext(nc) as tc:
        with (
            tc.tile_pool(name="const", bufs=1) as cpool,
            tc.tile_pool(name="pa", bufs=8) as ppool_a,
            tc.tile_pool(name="pd", bufs=8) as ppool_d,
            tc.tile_pool(name="outsb", bufs=2) as opool,
            tc.tile_pool(name="psum_s", bufs=5, space=bass.MemorySpace.PSUM) as spsum,
            tc.tile_pool(name="psum_o", bufs=1, space=bass.MemorySpace.PSUM) as opsum,
        ):
            q_sb = cpool.tile([128, NHEADS * TOK], bf16, tag="q", name="q_sb")
            k_sb = cpool.tile([128, NHEADS * ZL], bf16, tag="k", name="k_sb")
            v_sb = cpool.tile([128, NHEADS * ZT * NA], bf16, tag="v", name="v_sb")

            # chunked loads so head-0 compute starts after ~0.5 MiB, not 18 MiB
            # first: the k z-tiles and q half that gate the very first MMs
            nc.sync.dma_start(k_sb[:, 0:512], kT[0][:, 0:512])
            nc.sync.dma_start(q_sb[:, 0:512], qT[0][:, 0:512])
            nc.sync.dma_start(q_sb[:, 512:TOK], qT[0][:, 512:TOK])
            for h in range(NHEADS):
                nchunk = 8 if h == 0 else (2 if h == 1 else 1)
                if h > 0:
                    nc.sync.dma_start(q_sb[:, h * TOK : (h + 1) * TOK], qT[h])
                for ch in range(nchunk):
                    zlo, zhi = ch * ZL // nchunk, (ch + 1) * ZL // nchunk
                    if not (h == 0 and ch == 0):  # head0 chunk0 issued above
                        nc.sync.dma_start(
                            k_sb[:, h * ZL + zlo : h * ZL + zhi], kT[h][:, zlo:zhi]
                        )
                    alo, ahi = zlo // 128 * NA, zhi // 128 * NA
                    # SWDGE ring so V transfers overlap the K/Q HWDGE ring
                    nc.gpsimd.dma_start(
                        v_sb[:, h * ZT * NA + alo : h * ZT * NA + ahi],
                        vA[h][:, alo:ahi],
                    )

            SCH_A, SCH_B = 184.6650558, 16250.5  # 128/ln2, bf16 one-pattern
            NG = NHEADS * ZT  # 256 z-tiles over all heads
            p_halves = {}  # g -> [tileA, tileB]
            o_ps_by_head = {}

            def emit_score_half(g, j):
                # one-bank score tile + its exp; 5 PSUM tiles in flight
                h, zt = divmod(g, ZT)
                s_ps = spsum.tile([128, 512], f32, tag="s", name="s_ps")
                nc.tensor.matmul(
                    s_ps[:],
                    k_sb[:, h * ZL + zt * 128 : h * ZL + (zt + 1) * 128],
                    q_sb[:, h * TOK + j * 512 : h * TOK + (j + 1) * 512],
                )
                if (g + j) % 2 == 1:
                    # DVE Schraudolph exp: bf16 bits via affine map.
                    # separate per-engine p pools keep each engine's
                    # WAW buffer chain independent of the other's.
                    ph = ppool_d.tile([128, 512], bf16, tag="pd", name="pd")
                    nc.vector.tensor_scalar(
                        ph[:].bitcast(mybir.dt.int16),
                        s_ps[:],
                        SCH_A,
                        SCH_B,
                        mybir.AluOpType.mult,
                        mybir.AluOpType.add,
                    )
                else:
                    ph = ppool_a.tile([128, 512], bf16, tag="pa", name="pa")
                    nc.scalar.activation(
                        ph[:], s_ps[:], mybir.ActivationFunctionType.Exp
                    )
                p_halves.setdefault(g, [None, None])[j] = ph

            def emit_values_half(g, jhalf, tts=None):
                h, zt = divmod(g, ZT)
                if zt == 0 and jhalf == 0:
                    # 3 PSUM banks hold the 8 [128,129] O accumulators (3+3+2)
                    o_ps_by_head[h] = [
                        opsum.tile([128, 3 * NA], f32, tag="o0", name="o0"),
                        opsum.tile([128, 3 * NA], f32, tag="o1", name="o1"),
                        opsum.tile([128, 2 * NA], f32, tag="o2", name="o2"),
                    ]
                o_ps = o_ps_by_head[h]
                for tt in tts or range(jhalf * 4, jhalf * 4 + 4):
                    bank, slot = divmod(tt, 3)
                    nc.tensor.matmul(
                        o_ps[bank][:, slot * NA : (slot + 1) * NA],
                        p_halves[g][tt // 4][
                            :, (tt % 4) * 128 : (tt % 4 + 1) * 128
                        ],
                        v_sb[
                            :,
                            h * ZT * NA + zt * NA : h * ZT * NA + (zt + 1) * NA,
                        ],
                        # start=True clears has_written for the WHOLE
                        # psum bank, so only slot 0 of each bank may
                        # issue it; other slots overwrite-on-first-write
                        # via the per-element has_written bit.
                        start=(zt == 0 and slot == 0 and tt % 3 == 0),
                        stop=(zt == ZT - 1),
                    )

            def emit_out_bank(h, out_sb, bank, engine):
                # copy one o_ps bank to SBUF and DMA it out; emitted as soon
                # as that bank's last value MM exists so the tail overlaps
                o_ps = o_ps_by_head[h]
                lo = [0, 3 * NA, 6 * NA][bank]
                hi = [3 * NA, 6 * NA, 8 * NA][bank]
                if engine == "s":
                    nc.scalar.copy(out_sb[:, lo:hi], o_ps[bank][:])
                else:
                    nc.vector.tensor_copy(out_sb[:, lo:hi], o_ps[bank][:])
                nc.sync.dma_start(outp[h][:, lo:hi], out_sb[:, lo:hi])

            # HAM warmup: dummy MMs on scratch data during the initial DMA
            # wait so the first real MMs run at 2.4GHz (warm) not 1.2 (cold)
            warm_in = cpool.tile([128, 256], bf16, tag="wi", name="warm_in")
            nc.vector.memset(warm_in[:], 0)
            for w in range(20):
                s_ps = spsum.tile([128, 256], f32, tag="s", name="s_ps")
                nc.tensor.matmul(s_ps[:], warm_in[:, 0:128], warm_in[:])

            # software pipeline: score halves run 2.5 z-tiles ahead of the
            # value MMs (A-halves 3 ahead, B-halves 2 ahead — exactly filling
            # the 5 score banks), so the ~1us exp chain latency hides under
            # ~2us of queued TensorE work with no WAR stalls: each score
            # half's bank-WAR target is an exp that earlier queue entries
            # already wait on.
            emit_score_half(0, 0)
            emit_score_half(0, 1)
            emit_score_half(1, 0)
            emit_score_half(1, 1)
            emit_score_half(2, 0)
            for g in range(NG):
                h, zt = divmod(g, ZT)
                emit_values_half(g, 0)
                if g + 2 < NG:
                    emit_score_half(g + 2, 1)
                emit_values_half(g, 1)
                if g + 3 < NG:
                    emit_score_half(g + 3, 0)
                del p_halves[g]
                if zt == ZT - 1:
                    out_sb = opool.tile(
                        [128, TT * NA], f32, tag="osb", name="out_sb"
                    )
                    emit_out_bank(h, out_sb, 0, "s")
                    emit_out_bank(h, out_sb, 1, "v")
                    emit_out_bank(h, out_sb, 2, "s")

    nc.compile()
    return nc


def _shard_inputs(h, keys_w, values_w):
    bf = ml_dtypes.bfloat16
    hh = np.ascontiguousarray(h.reshape(TOK, MEMDIM))
    qT = np.ascontiguousarray(
        hh.reshape(TOK, NHEADS, HD).transpose(1, 2, 0)
    ).astype(bf)
    in_maps = []
    for c in range(NCORES):
        ks = keys_w[c * ZL : (c + 1) * ZL]  # [ZL, MEMDIM]
        kTc = np.ascontiguousarray(
            ks.reshape(ZL, NHEADS, HD).transpose(1, 2, 0)
        ).astype(bf)
        vs = values_w[:, c * ZL : (c + 1) * ZL]  # [MEMDIM, ZL]
        v5 = vs.reshape(NHEADS, HD, ZT, 128).transpose(0, 3, 2, 1)  # [h,p,zt,n]
        vAc = np.ones((NHEADS, 128, ZT, NA), np.float32)
        vAc[..., :HD] = v5
        vAc = np.ascontiguousarray(vAc.reshape(NHEADS, 128, ZT * NA)).astype(bf)
        in_maps.append({"qT": qT, "kT": kTc, "vA": vAc})
    return in_maps


def _combine(results):
    acc = np.zeros((NHEADS, 128, TT, NA), np.float64)
    for r in results:
        acc += r["outp"].reshape(NHEADS, 128, TT, NA).astype(np.float64)
    res = acc[..., :HD] / acc[..., HD][..., None]  # [h, p, tt, d]
    res = res.transpose(2, 1, 0, 3)  # [tt, p, h, d]
    return np.ascontiguousarray(
        res.reshape(TOK, MEMDIM).reshape(B, S, MEMDIM).astype(np.float32)
    )


def kernel(h, keys_w, values_w, _trace=False, _tmpdir=None):
    global _compiled
    if _compiled is None:
        _compiled = _build()
    from concourse import bass_utils

    in_maps = _shard_inputs(
        np.asarray(h, dtype=np.float32),
        np.asarray(keys_w, dtype=np.float32),
        np.asarray(values_w, dtype=np.float32),
    )
    res = bass_utils.run_bass_kernel_spmd(
        _compiled,
        in_maps,
        core_ids=list(range(NCORES)),
        trace=_trace,
        tmpdir=_tmpdir,
    )
    out = _combine(res.results)
    if _trace:
        return out, res
    return out



# revision 33
# speedup vs baseline: 1.0010x; 1.0010x over previous
"""Trainium2 Bass kernel for nn_KVMem (scatter_memory attention-to-memory).

Computation (per reference):
  q = h.reshape(B,S,8,128); k = keys_w.reshape(32768,8,128)
  w = softmax(einsum('bshd,zhd->bshz', q, k), axis=z)
  out = einsum('bshz,hdz->bshd', w, values_w.reshape(8,128,32768))

Strategy: shard the memory axis z (32768) across 8 cores (4096 each).
Each core computes, per head:
  S^T[z,tok] = K^T(tile).T @ Q^T          (TensorE, bf16)
  P^T = exp(S^T)                          (ScalarE, PSUM->SBUF bf16)
  O[tok, 0:128] += P^T(tok-tile).T @ V^T_aug[z, 0:129]  (TensorE, PSUM accum)
where V^T_aug has a 129th column of ones, so column 128 of the O
accumulator is sum_z exp(S) — the softmax denominator — for free.
Host sums partial (O, denom) over cores and divides.

No max-subtraction: scores are q.k with k ~ N(0, 1/1024) over d=128, so
|score| < ~3; exp is safely in fp32/bf16 range.

Performance structure (per core, ~247us vs 317us for the naive overlap):
- TensorE is the roofline: 2056 streamed columns per z-tile (2x512 score
  + 8x129 value MMs) x 256 z-tiles = 227us busy at 2.4GHz. fp8 DoubleRow
  would halve the value phase but measures 3.8% rel err (gate 2e-2).
- exp would serialize everything on ScalarE alone (256 x 1.1us = 294us),
  so each z-tile's scores are computed as two [128,512] single-bank PSUM
  tiles and the two exp halves alternate between ScalarE ACTIVATE and a
  VectorE Schraudolph bit-trick exp: i16 = round(s*128/ln2 + 16250.5) is
  the bf16 bit pattern of ~exp(s)*const; the constant bias cancels in
  the softmax division. 50/50 split -> each engine ~65% busy; ripple
  error ~1.8% on half the tiles -> 1.37e-2 overall (gate 2e-2).
- software pipelining: score MMs run 2 z-tiles ahead of the value MMs
  (5 PSUM score banks in flight), hiding the ~1us exp chain latency; the
  value MMs wait on the exp semaphores with <1% TensorE idle.
- per-engine p-half pools avoid cross-engine WAW coupling; o_ps->SBUF
  copies split over ScalarE/DVE; per-bank output DMAs shorten the tail;
  dummy warmup MMs during the initial DMA wait un-throttle the PE clock
  (HAM) before real work arrives.
"""

import sys

sys.path.insert(0, "/opt/trn_rl_repo")

import numpy as np
import ml_dtypes

NCORES = 8
MEMDIM, MEMSIZE, NHEADS = 1024, 32768, 8
B, S = 2, 512
TOK = B * S  # 1024
HD = MEMDIM // NHEADS  # 128
ZL = MEMSIZE // NCORES  # 4096 z per core
ZT = ZL // 128  # 32 z-tiles per core
TT = TOK // 128  # 8 token tiles
NA = HD + 1  # 129 = value dims + ones column

_compiled = None


def _build():
    import concourse.bass as bass
    import concourse.tile as tile
    from concourse import bacc, mybir

    nc = bacc.Bacc(
        "TRN2", target_bir_lowering=False, debug=False, num_devices=NCORES
    )
    bf16 = mybir.dt.bfloat16
    f32 = mybir.dt.float32

    qT = nc.dram_tensor("qT", [NHEADS, HD, TOK], bf16, kind="ExternalInput").ap()
    kT = nc.dram_tensor("kT", [NHEADS, HD, ZL], bf16, kind="ExternalInput").ap()
    vA = nc.dram_tensor(
        "vA", [NHEADS, 128, ZT * NA], bf16, kind="ExternalInput"
    ).ap()
    outp = nc.dram_tensor(
        "outp", [NHEADS, 128, TT * NA], f32, kind="ExternalOutput"
    ).ap()

    with tile.TileContext(nc) as tc:
        with (
            tc.tile_pool(name="const", bufs=1) as cpool,
            tc.tile_pool(name="pa", bufs=8) as ppool_a,
            tc.tile_pool(name="pd", bufs=8) as ppool_d,
            tc.tile_pool(name="outsb", bufs=2) as opool,
            tc.tile_pool(name="psum_s", bufs=5, space=bass.MemorySpace.PSUM) as spsum,
            tc.tile_pool(name="psum_o", bufs=1, space=bass.MemorySpace.PSUM) as opsum,
        ):
            q_sb = cpool.tile([128, NHEADS * TOK], bf16, tag="q", name="q_sb")
            k_sb = cpool.tile([128, NHEADS * ZL], bf16, tag="k", name="k_sb")
            v_sb = cpool.tile([128, NHEADS * ZT * NA], bf16, tag="v", name="v_sb")

            # chunked loads so head-0 compute starts after ~0.5 MiB, not 18 MiB
            # first: the k z-tiles and q half that gate the very first MMs
            nc.sync.dma_start(k_sb[:, 0:512], kT[0][:, 0:512])
            nc.sync.dma_start(q_sb[:, 0:512], qT[0][:, 0:512])
            nc.sync.dma_start(q_sb[:, 512:TOK], qT[0][:, 512:TOK])
            for h in range(NHEADS):
                nchunk = 8 if h == 0 else (2 if h == 1 else 1)
                if h > 0:
                    nc.sync.dma_start(q_sb[:, h * TOK : (h + 1) * TOK], qT[h])
                for ch in range(nchunk):
                    zlo, zhi = ch * ZL // nchunk, (ch + 1) * ZL // nchunk
                    if not (h == 0 and ch == 0):  # head0 chunk0 issued above
                        nc.sync.dma_start(
                            k_sb[:, h * ZL + zlo : h * ZL + zhi], kT[h][:, zlo:zhi]
                        )
                    alo, ahi = zlo // 128 * NA, zhi // 128 * NA
                    # SWDGE ring so V transfers overlap the K/Q HWDGE ring
                    nc.gpsimd.dma_start(
                        v_sb[:, h * ZT * NA + alo : h * ZT * NA + ahi],
                        vA[h][:, alo:ahi],
                    )

            SCH_A, SCH_B = 184.6650558, 16250.5  # 128/ln2, bf16 one-pattern
            NG = NHEADS * ZT  # 256 z-tiles over all heads
            p_halves = {}  # g -> [tileA, tileB]
            o_ps_by_head = {}

            def emit_score_half(g, j):
                # one-bank score tile + its exp; 5 PSUM tiles in flight
                h, zt = divmod(g, ZT)
                s_ps = spsum.tile([128, 512], f32, tag="s", name="s_ps")
                nc.tensor.matmul(
                    s_ps[:],
                    k_sb[:, h * ZL + zt * 128 : h * ZL + (zt + 1) * 128],
                    q_sb[:, h * TOK + j * 512 : h * TOK + (j + 1) * 512],
                )
                if (g + j) % 2 == 1:
                    # DVE Schraudolph exp: bf16 bits via affine map.
                    # separate per-engine p pools keep each engine's
                    # WAW buffer chain independent of the other's.
                    ph = ppool_d.tile([128, 512], bf16, tag="pd", name="pd")
                    nc.vector.tensor_scalar(
                        ph[:].bitcast(mybir.dt.int16),
                        s_ps[:],
                        SCH_A,
                        SCH_B,
                        mybir.AluOpType.mult,
                        mybir.AluOpType.add,
                    )
                else:
                    ph = ppool_a.tile([128, 512], bf16, tag="pa", name="pa")
                    nc.scalar.activation(
                        ph[:], s_ps[:], mybir.ActivationFunctionType.Exp
                    )
                p_halves.setdefault(g, [None, None])[j] = ph

            def emit_values_half(g, jhalf, tts=None):
                h, zt = divmod(g, ZT)
                if zt == 0 and jhalf == 0:
                    # 3 PSUM banks hold the 8 [128,129] O accumulators (3+3+2)
                    o_ps_by_head[h] = [
                        opsum.tile([128, 3 * NA], f32, tag="o0", name="o0"),
                        opsum.tile([128, 3 * NA], f32, tag="o1", name="o1"),
                        opsum.tile([128, 2 * NA], f32, tag="o2", name="o2"),
                    ]
                o_ps = o_ps_by_head[h]
                for tt in tts or range(jhalf * 4, jhalf * 4 + 4):
                    bank, slot = divmod(tt, 3)
                    nc.tensor.matmul(
                        o_ps[bank][:, slot * NA : (slot + 1) * NA],
                        p_halves[g][tt // 4][
                            :, (tt % 4) * 128 : (tt % 4 + 1) * 128
                        ],
                        v_sb[
                            :,
                            h * ZT * NA + zt * NA : h * ZT * NA + (zt + 1) * NA,
                        ],
                        # start=True clears has_written for the WHOLE
                        # psum bank, so only slot 0 of each bank may
                        # issue it; other slots overwrite-on-first-write
                        # via the per-element has_written bit.
                        start=(zt == 0 and slot == 0 and tt % 3 == 0),
                        stop=(zt == ZT - 1),
                    )

            def emit_out_bank(h, out_sb, bank, engine):
                # copy one o_ps bank to SBUF and DMA it out; emitted as soon
                # as that bank's last value MM exists so the tail overlaps
                o_ps = o_ps_by_head[h]
                lo = [0, 3 * NA, 6 * NA][bank]
                hi = [3 * NA, 6 * NA, 8 * NA][bank]
                if engine == "s":
                    nc.scalar.copy(out_sb[:, lo:hi], o_ps[bank][:])
                else:
                    nc.vector.tensor_copy(out_sb[:, lo:hi], o_ps[bank][:])
                nc.sync.dma_start(outp[h][:, lo:hi], out_sb[:, lo:hi])

            # HAM warmup: dummy MMs on scratch data during the initial DMA
            # wait so the first real MMs run at 2.4GHz (warm) not 1.2 (cold)
            warm_in = cpool.tile([128, 256], bf16, tag="wi", name="warm_in")
            nc.vector.memset(warm_in[:], 0)
            # dummy ACTIVATE pre-loads the ~2.7us exp spline table set during
            # the initial DMA wait, off the first real exp's critical path
            warm_p = ppool_a.tile([128, 512], bf16, tag="pa", name="pa")
            nc.scalar.activation(
                warm_p[:, 0:256], warm_in[:], mybir.ActivationFunctionType.Exp
            )
            for w in range(20):
                s_ps = spsum.tile([128, 256], f32, tag="s", name="s_ps")
                nc.tensor.matmul(s_ps[:], warm_in[:, 0:128], warm_in[:])

            # software pipeline: score halves run 2.5 z-tiles ahead of the
            # value MMs (A-halves 3 ahead, B-halves 2 ahead — exactly filling
            # the 5 score banks), so the ~1us exp chain latency hides under
            # ~2us of queued TensorE work with no WAR stalls: each score
            # half's bank-WAR target is an exp that earlier queue entries
            # already wait on.
            emit_score_half(0, 0)
            emit_score_half(0, 1)
            emit_score_half(1, 0)
            emit_score_half(1, 1)
            emit_score_half(2, 0)
            for g in range(NG):
                h, zt = divmod(g, ZT)
                emit_values_half(g, 0)
                if g + 2 < NG:
                    emit_score_half(g + 2, 1)
                emit_values_half(g, 1)
                if g + 3 < NG:
                    emit_score_half(g + 3, 0)
                del p_halves[g]
                if zt == ZT - 1:
                    out_sb = opool.tile(
                        [128, TT * NA], f32, tag="osb", name="out_sb"
                    )
                    emit_out_bank(h, out_sb, 0, "s")
                    emit_out_bank(h, out_sb, 1, "v")
                    emit_out_bank(h, out_sb, 2, "s")

    nc.compile()
    return nc


def _shard_inputs(h, keys_w, values_w):
    bf = ml_dtypes.bfloat16
    hh = np.ascontiguousarray(h.reshape(TOK, MEMDIM))
    qT = np.ascontiguousarray(
        hh.reshape(TOK, NHEADS, HD).transpose(1, 2, 0)
    ).astype(bf)
    in_maps = []
    for c in range(NCORES):
        ks = keys_w[c * ZL : (c + 1) * ZL]  # [ZL, MEMDIM]
        kTc = np.ascontiguousarray(
            ks.reshape(ZL, NHEADS, HD).transpose(1, 2, 0)
        ).astype(bf)
        vs = values_w[:, c * ZL : (c + 1) * ZL]  # [MEMDIM, ZL]
        v5 = vs.reshape(NHEADS, HD, ZT, 128).transpose(0, 3, 2, 1)  # [h,p,zt,n]
        vAc = np.ones((NHEADS, 128, ZT, NA), np.float32)
        vAc[..., :HD] = v5
        vAc = np.ascontiguousarray(vAc.reshape(NHEADS, 128, ZT * NA)).astype(bf)
        in_maps.append({"qT": qT, "kT": kTc, "vA": vAc})
    return in_maps


def _combine(results):
    acc = np.zeros((NHEADS, 128, TT, NA), np.float64)
    for r in results:
        acc += r["outp"].reshape(NHEADS, 128, TT, NA).astype(np.float64)
    res = acc[..., :HD] / acc[..., HD][..., None]  # [h, p, tt, d]
    res = res.transpose(2, 1, 0, 3)  # [tt, p, h, d]
    return np.ascontiguousarray(
        res.reshape(TOK, MEMDIM).reshape(B, S, MEMDIM).astype(np.float32)
    )


def kernel(h, keys_w, values_w, _trace=False, _tmpdir=None):
    global _compiled
    if _compiled is None:
        _compiled = _build()
    from concourse import bass_utils

    in_maps = _shard_inputs(
        np.asarray(h, dtype=np.float32),
        np.asarray(keys_w, dtype=np.float32),
        np.asarray(values_w, dtype=np.float32),
    )
    res = bass_utils.run_bass_kernel_spmd(
        _compiled,
        in_maps,
        core_ids=list(range(NCORES)),
        trace=_trace,
        tmpdir=_tmpdir,
    )
    out = _combine(res.results)
    if _trace:
        return out, res
    return out

